# revision 29
# baseline (speedup 1.0000x reference)
"""EqualizedModulatedConv2d (StyleGAN2) Trainium2 kernel.

Strategy: data-parallel over batch B=16 across 8 NeuronCores (2 samples/core).
Conv algorithm: F(4,3) Winograd along the x-dim (6 taps -> 4 output cols),
direct accumulation along y (3 dy taps folded into the PSUM accumulation),
fp16 matmul operands with fp32 PSUM accumulate.

Per core:
  1. style FC (PE, f32) -> es[i,b] = elr*(lin*(style @ fcW.T)[b,i] + fc_bias)
  2. demod norm from host-precomputed w2[i,o]: denom = elr^2 * (w2.T @ es^2),
     norm = rsqrt(denom + 1e-8)   (PE + Act + DVE, tiny)
  3. modulate: xm = x * es (Act engine, fp16), x shipped fp16,
     column-phase-deinterleaved (4 phases x 17) so Winograd input-transform
     reads are stride-1 (enables the DVE 4x perf mode)
  4. input transform: 12 scalar_tensor_tensor ops per (sample, half, icChunk)
     building V[6 taps][35 rows][16 x-tiles] fp16
  5. conv: per (sample, half, rowTile16, ocChunk): 6 taps x 3 dy x 4 ic = 72
     fp16 matmuls, free dim 256 (16 rows x 16 tiles), accumulating 6 tap
     planes in PSUM
  6. inverse transform A^T m: pair-sums on Pool engine, combines on DVE
     (fp16, 4x mode), demod scale + col re-interleave on Act, DMA out.

Host side: winograd weight transform U = G @ w (f64->fp16), w2 = sum(w^2),
x padding + phase deinterleave + fp16 cast, per-core batch sharding.
"""
import numpy as np

B, IC, OC, K, H, W, S = 16, 512, 512, 3, 64, 64, 512
NCORES = 8
BL = B // NCORES          # samples per core
ICC = IC // 128
OCC = OC // 128
SC = S // 128
NT = 6                    # winograd taps F(4,3)
XT = W // 4               # 16 x-tiles per row
NPH = 17                  # phase width (68 padded cols / 4 phases)
PW = 4 * NPH              # 68 padded width
HR = 35                   # rows per half (padded rows 0..34 / 31..65)
RT = 16                   # output rows per conv group
ELR = (2.0 / (IC * K * K)) ** 0.5
LIN = (2.0 / S) ** 0.5

_CACHE = {}

# F(4,3) winograd input transform B^T (host side)
_BT = np.array([
    [4, 0, -5, 0, 1, 0],
    [0, -4, -4, 1, 1, 0],
    [0, 4, -4, -1, 1, 0],
    [0, -2, -1, 2, 1, 0],
    [0, 2, -1, -2, 1, 0],
    [0, 4, 0, -5, 0, 1],
], dtype=np.float64)

# F(4,3) winograd weight transform (host side, f64)
_G = np.array([
    [1 / 4, 0, 0],
    [-1 / 6, -1 / 6, -1 / 6],
    [-1 / 6, 1 / 6, -1 / 6],
    [1 / 24, 1 / 12, 1 / 6],
    [1 / 24, -1 / 12, 1 / 6],
    [0, 0, 1],
], dtype=np.float64)


def _build():
    import concourse.bacc as bacc
    import concourse.mybir as mybir
    import concourse.tile as tile

    f32 = mybir.dt.float32
    f16 = mybir.dt.float16
    ALU = mybir.AluOpType
    AF = mybir.ActivationFunctionType

    nc = bacc.Bacc(None, target_bir_lowering=False, debug=False)
    xph = nc.dram_tensor("xph", [BL, ICC, 2, 128, NT * HR * XT], f16,
                         kind="ExternalInput").ap()
    ut = nc.dram_tensor("ut", [ICC, 128, OCC, 128 * K * NT], f16,
                        kind="ExternalInput").ap()
    esd = nc.dram_tensor("esd", [128, ICC * BL], f32, kind="ExternalInput").ap()
    normd = nc.dram_tensor("normd", [128, OCC * BL], f32,
                           kind="ExternalInput").ap()
    y = nc.dram_tensor("y", [BL, OC, H, W], f32, kind="ExternalOutput").ap()

    with tile.TileContext(nc) as tc:
        with (
            tc.tile_pool(name="up", bufs=1) as up,
            tc.tile_pool(name="sml", bufs=1) as sml,
            tc.tile_pool(name="xin", bufs=6) as xinp,
            tc.tile_pool(name="vp", bufs=2) as vp,
            tc.tile_pool(name="ivp", bufs=2) as ivp,
            tc.tile_pool(name="outp", bufs=3) as outp,
            tc.tile_pool(name="acc", bufs=2, space="PSUM") as accp,
        ):
            # ---- scalars (host-computed es / demod norm) ----
            es_sb = sml.tile([128, ICC, BL], f32)
            nc.sync.dma_start(es_sb.rearrange("p i b -> p (i b)"), esd)
            norm_sb = sml.tile([128, OCC, BL], f32)
            nc.sync.dma_start(norm_sb.rearrange("p o b -> p (o b)"), normd)

            # ---- first-step V taps on the fast sync queue ----
            xin0 = []
            for ic in range(ICC):
                xt = xinp.tile([128, NT * HR * XT], f16, tag="xin")
                nc.sync.dma_start(xt[:], xph[0, ic, 0])
                xin0.append(xt)

            # ---- U taps DMA: oc0/oc1 on sync, oc2/oc3 on gpsimd queue ----
            u_sb = up.tile([128, ICC, OCC, 128, K, NT], f16)
            for oc in range(OCC):
                q = nc.sync if oc < 2 else nc.gpsimd
                for ic in range(ICC):
                    q.dma_start(
                        u_sb[:, ic, oc].rearrange("p o a b -> p (o a b)"),
                        ut[ic, :, oc, :],
                    )

            # ---- V build: DMA host-pretransformed taps, modulate by es
            # (DVE tensor_scalar, 4x perf mode) ----
            def build_v(vtile, b, h, ic, xt=None):
                if xt is None:
                    xt = xinp.tile([128, NT * HR * XT], f16, tag="xin")
                    nc.scalar.dma_start(xt[:], xph[b, ic, h])
                vflat = vtile.rearrange("p i t r x -> p i (t r x)")
                nc.vector.tensor_scalar_mul(
                    vflat[:, ic], xt[:], es_sb[:, ic, b:b + 1],
                )

            # ---- conv group: 72 matmuls + inverse + demod + store ----
            def conv_group(vtile, b, h, rt, oc):
                ps = accp.tile([128, NT, RT * XT], f32, tag="acc")
                base = 16 * rt + (1 if h else 0)
                osl = slice(oc * 128, (oc + 1) * 128)
                for tap in range(NT):
                    pview = ps[:, tap, :]
                    for ic in range(ICC):
                        for dy in range(K):
                            r0 = base + dy
                            nc.tensor.matmul(
                                pview,
                                u_sb[:, ic, oc, :, dy, tap],
                                vtile[:, ic, tap, r0:r0 + RT, :].rearrange(
                                    "p r t -> p (r t)"),
                                start=(dy == 0 and ic == 0),
                                stop=(dy == K - 1 and ic == ICC - 1),
                            )
                # inverse transform: Act drains PSUM -> fp16 SBUF, then DVE
                # combines with 2x-mode tensor_tensor / 4x tensor_scalar ops
                c13 = ivp.tile([128, 2, RT * XT], f16, tag="c13")
                nc.scalar.copy(c13[:], ps[:, 1:5:2, :])
                c24 = ivp.tile([128, 2, RT * XT], f16, tag="c24")
                nc.scalar.copy(c24[:], ps[:, 2:6:2, :])
                c05 = ivp.tile([128, 2, RT * XT], f16, tag="c05")
                nc.scalar.copy(c05[:], ps[:, 0:6:5, :])
                PR = ivp.tile([128, 2, RT * XT], f16, tag="PR")
                QS = ivp.tile([128, 2, RT * XT], f16, tag="QS")
                nc.vector.tensor_add(PR[:], c13[:], c24[:])
                nc.vector.tensor_sub(QS[:], c13[:], c24[:])
                sc2 = ivp.tile([128, 2, RT * XT], f16, tag="sc2")
                nc.vector.tensor_add(sc2[:, 0], PR[:, 0], PR[:, 1])
                s8 = ivp.tile([128, 2, RT * XT], f16, tag="s8")
                nc.vector.tensor_scalar_mul(s8[:, 0], QS[:, 1], 8.0)
                nc.vector.tensor_add(sc2[:, 1], s8[:, 0], QS[:, 0])
                o03 = ivp.tile([128, 2, RT * XT], f16, tag="o03")
                nc.vector.tensor_add(o03[:], c05[:], sc2[:])
                o12 = ivp.tile([128, 2, RT * XT], f16, tag="o12")
                nc.vector.tensor_scalar_mul(s8[:, 1], QS[:, 1], 2.0)
                nc.vector.tensor_add(o12[:, 0], s8[:, 1], QS[:, 0])
                nc.vector.tensor_scalar_mul(s8[:, 0], PR[:, 1], 4.0)
                nc.vector.tensor_add(o12[:, 1], s8[:, 0], PR[:, 0])
                # demod scale + column re-interleave on Act
                ot = outp.tile([128, RT * W], f32, tag="ot")
                ov = ot.rearrange("p (r t four) -> p r t four", four=4, t=XT)
                nv = norm_sb[:, oc, b:b + 1]
                o03v = o03.rearrange("p two (r t) -> p two r t", t=XT)
                o12v = o12.rearrange("p two (r t) -> p two r t", t=XT)
                nc.scalar.mul(ov[:, :, :, 0], o03v[:, 0], nv)
                nc.scalar.mul(ov[:, :, :, 1], o12v[:, 0], nv)
                nc.scalar.mul(ov[:, :, :, 2], o12v[:, 1], nv)
                nc.scalar.mul(ov[:, :, :, 3], o03v[:, 1], nv)
                r0g = 32 * h + 16 * rt
                nc.sync.dma_start(
                    y[b, osl, r0g:r0g + RT, :].rearrange("p r c -> p (r c)"),
                    ot[:],
                )

            # ---- main pipeline ----
            steps = [(b, h) for b in range(BL) for h in range(2)]
            vtiles = []
            v0 = vp.tile([128, ICC, NT, HR, XT], f16, tag="V")
            for ic in range(ICC):
                build_v(v0, steps[0][0], steps[0][1], ic, xin0[ic])
            vtiles.append(v0)
            for i, (b, h) in enumerate(steps):
                vt = vtiles[i]
                if i + 1 < len(steps):
                    vnext = vp.tile([128, ICC, NT, HR, XT], f16, tag="V")
                    vtiles.append(vnext)
                    nb, nh = steps[i + 1]
                    pending = list(range(ICC))
                else:
                    vnext, pending = None, []
                gi = 0
                for rt in range(2):
                    for oc in range(OCC):
                        conv_group(vt, b, h, rt, oc)
                        if gi < len(pending):
                            build_v(vnext, nb, nh, pending[gi])
                        gi += 1
    nc.compile()
    return nc


class _Runner:
    """Persistent jitted PJRT executor for the SPMD kernel (axon path)."""

    def __init__(self, nc, n_cores):
        import jax
        import numpy as np
        from jax.sharding import Mesh, PartitionSpec
        try:
            from jax.experimental.shard_map import shard_map
        except ImportError:
            from jax.shard_map import shard_map
        import concourse.mybir as mybir
        from concourse.bass2jax import (
            _bass_exec_p, install_neuronx_cc_hook, partition_id_tensor,
        )

        install_neuronx_cc_hook()
        self.jax = jax
        self.n_cores = n_cores
        partition_name = (
            nc.partition_id_tensor.name if nc.partition_id_tensor else None
        )
        in_names, out_names, out_avals, zero_outs = [], [], [], []
        for alloc in nc.m.functions[0].allocations:
            if not isinstance(alloc, mybir.MemoryLocationSet):
                continue
            name = alloc.memorylocations[0].name
            if alloc.kind == "ExternalInput":
                if name != partition_name:
                    in_names.append(name)
            elif alloc.kind == "ExternalOutput":
                out_names.append(name)
                shape = tuple(alloc.tensor_shape)
                dtype = mybir.dt.np(alloc.dtype)
                out_avals.append(jax.core.ShapedArray(shape, dtype))
                zero_outs.append(np.zeros(shape, dtype))
        self.in_names, self.out_names, self.out_avals = in_names, out_names, out_avals

        def _body(*args):
            operands = list(args)
            if partition_name is not None:
                operands.append(partition_id_tensor())
            return tuple(
                _bass_exec_p.bind(
                    *operands,
                    out_avals=tuple(out_avals),
                    in_names=tuple(in_names + out_names + ([partition_name] if partition_name else [])),
                    out_names=tuple(out_names),
                    lowering_input_output_aliases=(),
                    sim_require_finite=False,
                    sim_require_nnan=False,
                    nc=nc,
                )
            )

        devices = jax.devices()[:n_cores]
        mesh = Mesh(np.asarray(devices), ("core",))
        n_params = len(in_names)
        self.fn = jax.jit(
            shard_map(
                _body, mesh=mesh,
                in_specs=(PartitionSpec("core"),) * (n_params + len(out_names)),
                out_specs=(PartitionSpec("core"),) * len(out_names),
                check_rep=False,
            ),
            keep_unused=True,
        )
        self.sharding = jax.sharding.NamedSharding(mesh, PartitionSpec("core"))
        self._dev_zeros = [
            jax.device_put(
                np.zeros((n_cores * z.shape[0], *z.shape[1:]), z.dtype), self.sharding
            )
            for z in zero_outs
        ]

    def put_inputs(self, in_maps):
        concat = [
            np.concatenate(
                [np.asarray(in_maps[c][n]) for c in range(self.n_cores)], axis=0
            )
            for n in self.in_names
        ]
        return [self.jax.device_put(a, self.sharding) for a in concat]

    def run(self, dev_args):
        outs = self.fn(*dev_args, *self._dev_zeros)
        self.jax.block_until_ready(outs)
        return outs

    def results(self, outs):
        res = []
        for c in range(self.n_cores):
            d = {}
            for i, name in enumerate(self.out_names):
                full = np.asarray(outs[i])
                d[name] = full.reshape(self.n_cores, *self.out_avals[i].shape)[c]
            res.append(d)
        return res


def _get_runner():
    if "runner" not in _CACHE:
        nc = _build()
        _CACHE["nc"] = nc
        _CACHE["runner"] = _Runner(nc, NCORES)
    return _CACHE["runner"]


def _prep_inputs(x, style, weight, fc_weight, fc_bias):
    """Host-side sharding + layout marshalling. Returns per-core input maps."""
    x = np.asarray(x, dtype=np.float32)
    style = np.asarray(style, dtype=np.float32)
    weight = np.asarray(weight, dtype=np.float32)
    fc_weight = np.asarray(fc_weight, dtype=np.float32)
    fc_bias = np.asarray(fc_bias, dtype=np.float32)

    # winograd weight taps U[i, o, dy, tap] (f64 transform, fp16 ship)
    U = np.einsum("tk,oidk->iodt", _G, weight.astype(np.float64))
    ut_host = np.ascontiguousarray(
        U.reshape(ICC, 128, OCC, 128, K * NT)
        .transpose(0, 1, 2, 3, 4)
        .reshape(ICC, 128, OCC, 128 * K * NT)
        .astype(np.float16)
    )
    # style FC + demod norm on host (f64): es = elr*s, norm = rsqrt(denom+eps)
    s = (style.astype(np.float64) * LIN) @ fc_weight.astype(np.float64).T \
        + fc_bias.astype(np.float64)                       # [B, IC]
    es = (ELR * s).astype(np.float32)
    w2 = (weight.astype(np.float64) ** 2).sum(axis=(2, 3))  # [oC, iC]
    denom = (ELR * ELR) * np.einsum("oi,bi->bo", w2, s * s)
    norm = (1.0 / np.sqrt(denom + 1e-8)).astype(np.float32)  # [B, OC]

    # x: pad to 66 rows x 68 cols, winograd F(4,3) input transform along x
    # (host, f32), fp16, split into two 35-row halves
    xpad = np.zeros((B, IC, H + 2, PW), dtype=np.float32)
    xpad[:, :, 1:H + 1, 1:W + 1] = x
    cols = 4 * np.arange(XT)
    d = np.stack([xpad[:, :, :, cols + k] for k in range(NT)], axis=2)
    # V[b, i, tap, row, xtile] = sum_k BT[tap, k] * d[b, i, k, row, xtile]
    V = np.einsum("tk,bikrx->bitrx", _BT.astype(np.float32), d)
    Vr = V.reshape(B, ICC, 128, NT, H + 2, XT)
    halves = np.stack([Vr[:, :, :, :, 0:HR], Vr[:, :, :, :, 31:66]], axis=3)
    xph_host = np.ascontiguousarray(
        halves.transpose(0, 1, 3, 2, 4, 5, 6)
        .reshape(B, ICC, 2, 128, NT * HR * XT)
        .astype(np.float16)
    )

    in_maps = []
    for c in range(NCORES):
        sl = slice(c * BL, (c + 1) * BL)
        in_maps.append({
            "xph": np.ascontiguousarray(xph_host[sl]),
            "ut": ut_host,
            "esd": np.ascontiguousarray(
                es[sl].T.reshape(ICC, 128, BL).transpose(1, 0, 2)
                .reshape(128, ICC * BL)
            ),
            "normd": np.ascontiguousarray(
                norm[sl].T.reshape(OCC, 128, BL).transpose(1, 0, 2)
                .reshape(128, OCC * BL)
            ),
        })
    return in_maps


def kernel(x, style, weight, fc_weight, fc_bias):
    runner = _get_runner()
    in_maps = _prep_inputs(x, style, weight, fc_weight, fc_bias)
    dev_args = runner.put_inputs(in_maps)
    outs = runner.run(dev_args)
    res = runner.results(outs)
    out = np.concatenate([res[c]["y"] for c in range(NCORES)], axis=0)
    return out.astype(np.float32)


# revision 30
# speedup vs baseline: 1.0411x; 1.0411x over previous
"""EqualizedModulatedConv2d (StyleGAN2) Trainium2 kernel.

Strategy: data-parallel over batch B=16 across 8 NeuronCores (2 samples/core).
Conv algorithm: F(4,3) Winograd along the x-dim (6 taps -> 4 output cols),
direct accumulation along y (3 dy taps folded into the PSUM accumulation),
fp16 matmul operands with fp32 PSUM accumulate.

Per core:
  1. style FC (PE, f32) -> es[i,b] = elr*(lin*(style @ fcW.T)[b,i] + fc_bias)
  2. demod norm from host-precomputed w2[i,o]: denom = elr^2 * (w2.T @ es^2),
     norm = rsqrt(denom + 1e-8)   (PE + Act + DVE, tiny)
  3. modulate: xm = x * es (Act engine, fp16), x shipped fp16,
     column-phase-deinterleaved (4 phases x 17) so Winograd input-transform
     reads are stride-1 (enables the DVE 4x perf mode)
  4. input transform: 12 scalar_tensor_tensor ops per (sample, half, icChunk)
     building V[6 taps][35 rows][16 x-tiles] fp16
  5. conv: per (sample, half, rowTile16, ocChunk): 6 taps x 3 dy x 4 ic = 72
     fp16 matmuls, free dim 256 (16 rows x 16 tiles), accumulating 6 tap
     planes in PSUM
  6. inverse transform A^T m: pair-sums on Pool engine, combines on DVE
     (fp16, 4x mode), demod scale + col re-interleave on Act, DMA out.

Host side: winograd weight transform U = G @ w (f64->fp16), w2 = sum(w^2),
x padding + phase deinterleave + fp16 cast, per-core batch sharding.
"""
import numpy as np

B, IC, OC, K, H, W, S = 16, 512, 512, 3, 64, 64, 512
NCORES = 8
BL = B // NCORES          # samples per core
ICC = IC // 128
OCC = OC // 128
SC = S // 128
NT = 6                    # winograd taps F(4,3)
XT = W // 4               # 16 x-tiles per row
NPH = 17                  # phase width (68 padded cols / 4 phases)
PW = 4 * NPH              # 68 padded width
HR = 35                   # rows per half (padded rows 0..34 / 31..65)
RT = 16                   # output rows per conv group
ELR = (2.0 / (IC * K * K)) ** 0.5
LIN = (2.0 / S) ** 0.5

_CACHE = {}

# F(4,3) winograd input transform B^T (host side)
_BT = np.array([
    [4, 0, -5, 0, 1, 0],
    [0, -4, -4, 1, 1, 0],
    [0, 4, -4, -1, 1, 0],
    [0, -2, -1, 2, 1, 0],
    [0, 2, -1, -2, 1, 0],
    [0, 4, 0, -5, 0, 1],
], dtype=np.float64)

# F(4,3) winograd weight transform (host side, f64)
_G = np.array([
    [1 / 4, 0, 0],
    [-1 / 6, -1 / 6, -1 / 6],
    [-1 / 6, 1 / 6, -1 / 6],
    [1 / 24, 1 / 12, 1 / 6],
    [1 / 24, -1 / 12, 1 / 6],
    [0, 0, 1],
], dtype=np.float64)


def _build():
    import concourse.bacc as bacc
    import concourse.mybir as mybir
    import concourse.tile as tile

    f32 = mybir.dt.float32
    f16 = mybir.dt.float16
    ALU = mybir.AluOpType
    AF = mybir.ActivationFunctionType

    nc = bacc.Bacc(None, target_bir_lowering=False, debug=False)
    xph = nc.dram_tensor("xph", [BL, ICC, 2, 128, NT * HR * XT], f16,
                         kind="ExternalInput").ap()
    ut = nc.dram_tensor("ut", [ICC, 128, OCC, 128 * K * NT], f16,
                        kind="ExternalInput").ap()
    esd = nc.dram_tensor("esd", [128, ICC * BL], f32, kind="ExternalInput").ap()
    normd = nc.dram_tensor("normd", [128, OCC * BL], f32,
                           kind="ExternalInput").ap()
    y = nc.dram_tensor("y", [BL, OC, H, W], f32, kind="ExternalOutput").ap()

    with tile.TileContext(nc) as tc:
        with (
            tc.tile_pool(name="up", bufs=1) as up,
            tc.tile_pool(name="sml", bufs=1) as sml,
            tc.tile_pool(name="xin", bufs=6) as xinp,
            tc.tile_pool(name="vp", bufs=2) as vp,
            tc.tile_pool(name="ivp", bufs=2) as ivp,
            tc.tile_pool(name="outp", bufs=3) as outp,
            tc.tile_pool(name="acc", bufs=2, space="PSUM") as accp,
        ):
            # ---- scalars (host-computed es / demod norm) ----
            es_sb = sml.tile([128, ICC, BL], f32)
            nc.sync.dma_start(es_sb.rearrange("p i b -> p (i b)"), esd)
            norm_sb = sml.tile([128, OCC, BL], f32)
            nc.sync.dma_start(norm_sb.rearrange("p o b -> p (o b)"), normd)

            # ---- startup DMAs spread across the three queues: the first
            # conv group needs V[ic0..3] + U[ic0..3, oc0] ASAP ----
            xin0 = []
            for ic in range(ICC):
                xt = xinp.tile([128, NT * HR * XT], f16, tag="xin")
                q = nc.sync if ic < 2 else nc.scalar
                q.dma_start(xt[:], xph[0, ic, 0])
                xin0.append(xt)

            u_sb = up.tile([128, ICC, OCC, 128, K, NT], f16)

            def load_u(ic, oc, q):
                q.dma_start(
                    u_sb[:, ic, oc].rearrange("p o a b -> p (o a b)"),
                    ut[ic, :, oc, :],
                )

            load_u(0, 0, nc.gpsimd)
            load_u(1, 0, nc.gpsimd)
            load_u(2, 0, nc.sync)
            load_u(3, 0, nc.sync)
            for oc in range(1, OCC):
                for ic in range(ICC):
                    load_u(ic, oc, nc.gpsimd)

            # ---- V build: DMA host-pretransformed taps, modulate by es
            # (DVE tensor_scalar, 4x perf mode) ----
            def build_v(vtile, b, h, ic, xt=None):
                if xt is None:
                    xt = xinp.tile([128, NT * HR * XT], f16, tag="xin")
                    nc.scalar.dma_start(xt[:], xph[b, ic, h])
                vflat = vtile.rearrange("p i t r x -> p i (t r x)")
                nc.vector.tensor_scalar_mul(
                    vflat[:, ic], xt[:], es_sb[:, ic, b:b + 1],
                )

            # ---- conv group: 72 matmuls + inverse + demod + store ----
            def conv_group(vtile, b, h, rt, oc):
                ps = accp.tile([128, NT, RT * XT], f32, tag="acc")
                base = 16 * rt + (1 if h else 0)
                osl = slice(oc * 128, (oc + 1) * 128)
                for tap in range(NT):
                    pview = ps[:, tap, :]
                    for ic in range(ICC):
                        for dy in range(K):
                            r0 = base + dy
                            nc.tensor.matmul(
                                pview,
                                u_sb[:, ic, oc, :, dy, tap],
                                vtile[:, ic, tap, r0:r0 + RT, :].rearrange(
                                    "p r t -> p (r t)"),
                                start=(dy == 0 and ic == 0),
                                stop=(dy == K - 1 and ic == ICC - 1),
                            )
                # inverse transform: Act drains PSUM -> fp16 SBUF, then DVE
                # combines with 2x-mode tensor_tensor / 4x tensor_scalar ops
                c13 = ivp.tile([128, 2, RT * XT], f16, tag="c13")
                nc.scalar.copy(c13[:], ps[:, 1:5:2, :])
                c24 = ivp.tile([128, 2, RT * XT], f16, tag="c24")
                nc.scalar.copy(c24[:], ps[:, 2:6:2, :])
                c05 = ivp.tile([128, 2, RT * XT], f16, tag="c05")
                nc.scalar.copy(c05[:], ps[:, 0:6:5, :])
                PR = ivp.tile([128, 2, RT * XT], f16, tag="PR")
                QS = ivp.tile([128, 2, RT * XT], f16, tag="QS")
                nc.vector.tensor_add(PR[:], c13[:], c24[:])
                nc.vector.tensor_sub(QS[:], c13[:], c24[:])
                sc2 = ivp.tile([128, 2, RT * XT], f16, tag="sc2")
                nc.vector.tensor_add(sc2[:, 0], PR[:, 0], PR[:, 1])
                s8 = ivp.tile([128, 2, RT * XT], f16, tag="s8")
                nc.vector.tensor_scalar_mul(s8[:, 0], QS[:, 1], 8.0)
                nc.vector.tensor_add(sc2[:, 1], s8[:, 0], QS[:, 0])
                o03 = ivp.tile([128, 2, RT * XT], f16, tag="o03")
                nc.vector.tensor_add(o03[:], c05[:], sc2[:])
                o12 = ivp.tile([128, 2, RT * XT], f16, tag="o12")
                nc.vector.tensor_scalar_mul(s8[:, 1], QS[:, 1], 2.0)
                nc.vector.tensor_add(o12[:, 0], s8[:, 1], QS[:, 0])
                nc.vector.tensor_scalar_mul(s8[:, 0], PR[:, 1], 4.0)
                nc.vector.tensor_add(o12[:, 1], s8[:, 0], PR[:, 0])
                # demod scale + column re-interleave on Act
                ot = outp.tile([128, RT * W], f32, tag="ot")
                ov = ot.rearrange("p (r t four) -> p r t four", four=4, t=XT)
                nv = norm_sb[:, oc, b:b + 1]
                o03v = o03.rearrange("p two (r t) -> p two r t", t=XT)
                o12v = o12.rearrange("p two (r t) -> p two r t", t=XT)
                nc.scalar.mul(ov[:, :, :, 0], o03v[:, 0], nv)
                nc.scalar.mul(ov[:, :, :, 1], o12v[:, 0], nv)
                nc.scalar.mul(ov[:, :, :, 2], o12v[:, 1], nv)
                nc.scalar.mul(ov[:, :, :, 3], o03v[:, 1], nv)
                r0g = 32 * h + 16 * rt
                nc.sync.dma_start(
                    y[b, osl, r0g:r0g + RT, :].rearrange("p r c -> p (r c)"),
                    ot[:],
                )

            # ---- main pipeline ----
            steps = [(b, h) for b in range(BL) for h in range(2)]
            vtiles = []
            v0 = vp.tile([128, ICC, NT, HR, XT], f16, tag="V")
            for ic in range(ICC):
                build_v(v0, steps[0][0], steps[0][1], ic, xin0[ic])
            vtiles.append(v0)
            for i, (b, h) in enumerate(steps):
                vt = vtiles[i]
                if i + 1 < len(steps):
                    vnext = vp.tile([128, ICC, NT, HR, XT], f16, tag="V")
                    vtiles.append(vnext)
                    nb, nh = steps[i + 1]
                    pending = list(range(ICC))
                else:
                    vnext, pending = None, []
                gi = 0
                for rt in range(2):
                    for oc in range(OCC):
                        conv_group(vt, b, h, rt, oc)
                        if gi < len(pending):
                            build_v(vnext, nb, nh, pending[gi])
                        gi += 1
    nc.compile()
    return nc


class _Runner:
    """Persistent jitted PJRT executor for the SPMD kernel (axon path)."""

    def __init__(self, nc, n_cores):
        import jax
        import numpy as np
        from jax.sharding import Mesh, PartitionSpec
        try:
            from jax.experimental.shard_map import shard_map
        except ImportError:
            from jax.shard_map import shard_map
        import concourse.mybir as mybir
        from concourse.bass2jax import (
            _bass_exec_p, install_neuronx_cc_hook, partition_id_tensor,
        )

        install_neuronx_cc_hook()
        self.jax = jax
        self.n_cores = n_cores
        partition_name = (
            nc.partition_id_tensor.name if nc.partition_id_tensor else None
        )
        in_names, out_names, out_avals, zero_outs = [], [], [], []
        for alloc in nc.m.functions[0].allocations:
            if not isinstance(alloc, mybir.MemoryLocationSet):
                continue
            name = alloc.memorylocations[0].name
            if alloc.kind == "ExternalInput":
                if name != partition_name:
                    in_names.append(name)
            elif alloc.kind == "ExternalOutput":
                out_names.append(name)
                shape = tuple(alloc.tensor_shape)
                dtype = mybir.dt.np(alloc.dtype)
                out_avals.append(jax.core.ShapedArray(shape, dtype))
                zero_outs.append(np.zeros(shape, dtype))
        self.in_names, self.out_names, self.out_avals = in_names, out_names, out_avals

        def _body(*args):
            operands = list(args)
            if partition_name is not None:
                operands.append(partition_id_tensor())
            return tuple(
                _bass_exec_p.bind(
                    *operands,
                    out_avals=tuple(out_avals),
                    in_names=tuple(in_names + out_names + ([partition_name] if partition_name else [])),
                    out_names=tuple(out_names),
                    lowering_input_output_aliases=(),
                    sim_require_finite=False,
                    sim_require_nnan=False,
                    nc=nc,
                )
            )

        devices = jax.devices()[:n_cores]
        mesh = Mesh(np.asarray(devices), ("core",))
        n_params = len(in_names)
        self.fn = jax.jit(
            shard_map(
                _body, mesh=mesh,
                in_specs=(PartitionSpec("core"),) * (n_params + len(out_names)),
                out_specs=(PartitionSpec("core"),) * len(out_names),
                check_rep=False,
            ),
            keep_unused=True,
        )
        self.sharding = jax.sharding.NamedSharding(mesh, PartitionSpec("core"))
        self._dev_zeros = [
            jax.device_put(
                np.zeros((n_cores * z.shape[0], *z.shape[1:]), z.dtype), self.sharding
            )
            for z in zero_outs
        ]

    def put_inputs(self, in_maps):
        concat = [
            np.concatenate(
                [np.asarray(in_maps[c][n]) for c in range(self.n_cores)], axis=0
            )
            for n in self.in_names
        ]
        return [self.jax.device_put(a, self.sharding) for a in concat]

    def run(self, dev_args):
        outs = self.fn(*dev_args, *self._dev_zeros)
        self.jax.block_until_ready(outs)
        return outs

    def results(self, outs):
        res = []
        for c in range(self.n_cores):
            d = {}
            for i, name in enumerate(self.out_names):
                full = np.asarray(outs[i])
                d[name] = full.reshape(self.n_cores, *self.out_avals[i].shape)[c]
            res.append(d)
        return res


def _get_runner():
    if "runner" not in _CACHE:
        nc = _build()
        _CACHE["nc"] = nc
        _CACHE["runner"] = _Runner(nc, NCORES)
    return _CACHE["runner"]


def _prep_inputs(x, style, weight, fc_weight, fc_bias):
    """Host-side sharding + layout marshalling. Returns per-core input maps."""
    x = np.asarray(x, dtype=np.float32)
    style = np.asarray(style, dtype=np.float32)
    weight = np.asarray(weight, dtype=np.float32)
    fc_weight = np.asarray(fc_weight, dtype=np.float32)
    fc_bias = np.asarray(fc_bias, dtype=np.float32)

    # winograd weight taps U[i, o, dy, tap] (f64 transform, fp16 ship)
    U = np.einsum("tk,oidk->iodt", _G, weight.astype(np.float64))
    ut_host = np.ascontiguousarray(
        U.reshape(ICC, 128, OCC, 128, K * NT)
        .transpose(0, 1, 2, 3, 4)
        .reshape(ICC, 128, OCC, 128 * K * NT)
        .astype(np.float16)
    )
    # style FC + demod norm on host (f64): es = elr*s, norm = rsqrt(denom+eps)
    s = (style.astype(np.float64) * LIN) @ fc_weight.astype(np.float64).T \
        + fc_bias.astype(np.float64)                       # [B, IC]
    es = (ELR * s).astype(np.float32)
    w2 = (weight.astype(np.float64) ** 2).sum(axis=(2, 3))  # [oC, iC]
    denom = (ELR * ELR) * np.einsum("oi,bi->bo", w2, s * s)
    norm = (1.0 / np.sqrt(denom + 1e-8)).astype(np.float32)  # [B, OC]

    # x: pad to 66 rows x 68 cols, winograd F(4,3) input transform along x
    # (host, f32), fp16, split into two 35-row halves
    xpad = np.zeros((B, IC, H + 2, PW), dtype=np.float32)
    xpad[:, :, 1:H + 1, 1:W + 1] = x
    cols = 4 * np.arange(XT)
    d = np.stack([xpad[:, :, :, cols + k] for k in range(NT)], axis=2)
    # V[b, i, tap, row, xtile] = sum_k BT[tap, k] * d[b, i, k, row, xtile]
    V = np.einsum("tk,bikrx->bitrx", _BT.astype(np.float32), d)
    Vr = V.reshape(B, ICC, 128, NT, H + 2, XT)
    halves = np.stack([Vr[:, :, :, :, 0:HR], Vr[:, :, :, :, 31:66]], axis=3)
    xph_host = np.ascontiguousarray(
        halves.transpose(0, 1, 3, 2, 4, 5, 6)
        .reshape(B, ICC, 2, 128, NT * HR * XT)
        .astype(np.float16)
    )

    in_maps = []
    for c in range(NCORES):
        sl = slice(c * BL, (c + 1) * BL)
        in_maps.append({
            "xph": np.ascontiguousarray(xph_host[sl]),
            "ut": ut_host,
            "esd": np.ascontiguousarray(
                es[sl].T.reshape(ICC, 128, BL).transpose(1, 0, 2)
                .reshape(128, ICC * BL)
            ),
            "normd": np.ascontiguousarray(
                norm[sl].T.reshape(OCC, 128, BL).transpose(1, 0, 2)
                .reshape(128, OCC * BL)
            ),
        })
    return in_maps


def kernel(x, style, weight, fc_weight, fc_bias):
    runner = _get_runner()
    in_maps = _prep_inputs(x, style, weight, fc_weight, fc_bias)
    dev_args = runner.put_inputs(in_maps)
    outs = runner.run(dev_args)
    res = runner.results(outs)
    out = np.concatenate([res[c]["y"] for c in range(NCORES)], axis=0)
    return out.astype(np.float32)
